# revision 29
# baseline (speedup 1.0000x reference)
"""ACE loss kernel for TRN2, data-parallel over 8 NeuronCores.

Math (per sample b, with targets y[b, 0:8] and logits x[b, c, t]):
  m[b,t]   = max_c x[b,c,t]
  cnt[b,j] = #{t : x[b, y[b,j], t] == m[b,t]}        == n_k[b, y[b,j]] (no ties)
  dup[b,j] = multiplicity of y[b,j] within y[b,:]    == y_k[b, y[b,j]]
Only target classes contribute to the masked loss, so the full 128-bin
argmax histogram is never materialized:
  n_sum[b] = sum_j cnt/dup   (each distinct class counted once)
  n_p[b,j] = max(cnt / max(n_sum,1), EPS)
  loss[b]  = sum_j n_p * (-log(dup/8)) / dup
  out      = mean_b loss

Each core gets 1024 samples; its x shard (33.5 MB) is streamed through
SBUF in eight tiles (sample on partition, [class, t] on the free axis).
The class-max is computed by a pairwise max tree: the first tree level
runs inside the load DMA itself (second half of each tile is DMA'd with
accum_op=max onto the first half via the SDMA CCE unit), the remaining
levels are stride-1 VectorE tensor-tensor maxes (the strided
reduce_max runs at ~1.67 cyc/elem; stride-1 TT max at 1.0).  The 8
target-class rows per sample are host-pre-gathered (0.75% of input
bytes - pure index plumbing) and compared against the max on device.
All [128, 8]-sized loss math is batched across tiles into single
[128, 64] instructions.  Each core returns 128 partial loss sums; the
host adds them and divides by B.
"""

import numpy as np

B, C, T, L = 8192, 128, 64, 8
N_CORES = 8
B_SH = B // N_CORES          # 1024 samples per core
NT = B_SH // 128             # 8 tiles of 128 samples
EPS = 1e-5

_CACHE = {}


def _build_nc():
    import os
    import sys
    if "/opt/trn_rl_repo" not in sys.path:
        sys.path.insert(0, "/opt/trn_rl_repo")
    from concourse import bacc, mybir
    from concourse.tile import TileContext

    f32 = mybir.dt.float32
    AX = mybir.AxisListType
    OP = mybir.AluOpType

    # "gptree" (TT max on GpSimd) fails walrus codegen - Pool has no TT.
    # default "h16hw": x streams in as f32 on the HWDGE queue (SWDGE cast
    # loads add Q7 descriptor-gen latency); tree level 1 is a TT max with
    # f32 inputs and fp16 output (rounding is monotone, so max-then-round
    # == round-then-max), and the remaining levels run in fp16 at DVE 2x.
    # fp16 max-ties overcount slightly: measured 1.2e-4 relative loss
    # error vs the exact-f32 tree ("f32tree" variant).
    variant = set(os.environ.get("ACE_VARIANT", "h16hw").split(","))
    cdt = f32 if "f32tree" in variant else mybir.dt.float16

    nc = bacc.Bacc("TRN2")
    x = nc.declare_dram_parameter("x", [B_SH, C * T], f32, isOutput=False)
    # host-pre-gathered target rows, laid out [p, (tile, slot, t)]
    xg = nc.declare_dram_parameter("xg", [128, NT * L * T], cdt, isOutput=False)
    # target classes, laid out [p, (tile, slot)]
    yc = nc.declare_dram_parameter("yc", [128, NT * L], mybir.dt.int32, isOutput=False)
    out = nc.declare_dram_parameter("out", [128, 1], f32, isOutput=True)

    with TileContext(nc) as tc:
        with (
            tc.tile_pool(name="xp", bufs=4) as xp,
            tc.tile_pool(name="sp", bufs=2) as sp,
            tc.tile_pool(name="cp", bufs=1) as cp,
        ):
            # whole-core tiles; ycta/xga ride the scalar-engine HWDGE queue
            # so the sync queue starts streaming x tiles immediately
            # (xg arrives from the host already in the compare dtype)
            xga = cp.tile([128, NT * L * T], cdt)
            ycta = cp.tile([128, NT * L], mybir.dt.int32)
            nc.scalar.dma_start(out=ycta[:, :], in_=yc[:, :])
            nc.scalar.dma_start(out=xga[:, :], in_=xg[:, :])
            cnta = cp.tile([128, NT * L], f32)

            # ---- y-side math, hoisted before the loop: runs on DVE/ACT while
            # the first x tiles are still loading ----
            ycf = cp.tile([128, NT * L], f32)
            nc.scalar.copy(out=ycf[:, :], in_=ycta[:, :])
            # dup[p, (k, a)] = multiplicity of class a within its sample
            eq8 = cp.tile([128, NT * L * L], f32)
            nc.vector.tensor_tensor(
                out=eq8[:, :].rearrange("p (k a b) -> p k a b", a=L, b=L),
                in0=ycf[:, :].rearrange("p (k a) -> p k a", a=L)
                .unsqueeze(3).to_broadcast([128, NT, L, L]),
                in1=ycf[:, :].rearrange("p (k a) -> p k a", a=L)
                .unsqueeze(2).to_broadcast([128, NT, L, L]),
                op=OP.is_equal,
            )
            dup = cp.tile([128, NT * L], f32)
            nc.vector.reduce_sum(
                out=dup[:, :],
                in_=eq8[:, :].rearrange("p (k a b) -> p k a b", a=L, b=L),
                axis=AX.X,
            )
            rd = cp.tile([128, NT * L], f32)
            nc.vector.reciprocal(out=rd[:, :], in_=dup[:, :])
            lg = cp.tile([128, NT * L], f32)
            nc.scalar.activation(
                out=lg[:, :], in_=dup[:, :],
                func=mybir.ActivationFunctionType.Ln, scale=1.0 / L,
            )
            # wgt = -log(dup/8) / dup
            wgt = cp.tile([128, NT * L], f32)
            nc.vector.scalar_tensor_tensor(
                out=wgt[:, :], in0=lg[:, :], scalar=-1.0, in1=rd[:, :],
                op0=OP.mult, op1=OP.mult,
            )

            q = C * T // 4
            for k in range(NT):
                row = slice(k * 128, (k + 1) * 128)
                xt = xp.tile([128, C * T], f32, tag="xt")
                # quarter loads (order 0,2,1,3) so tree level 1 can start
                # after half the tile's bytes have landed
                for c in (0, 2, 1, 3):
                    nc.sync.dma_start(
                        out=xt[:, c * q:(c + 1) * q],
                        in_=x[row, c * q:(c + 1) * q],
                    )
                # level 1 as two halves, fusing the f32->cdt cast into the op
                mt = sp.tile([128, 2 * q], cdt, tag="mt")
                nc.vector.tensor_tensor(
                    out=mt[:, :q], in0=xt[:, :q],
                    in1=xt[:, 2 * q:3 * q], op=OP.max,
                )
                nc.vector.tensor_tensor(
                    out=mt[:, q:2 * q], in0=xt[:, q:2 * q],
                    in1=xt[:, 3 * q:4 * q], op=OP.max,
                )
                # remaining class-max levels: in-place stride-1 TT max tree
                w = 2 * q
                while w > T:
                    h = w // 2
                    nc.vector.tensor_tensor(
                        out=mt[:, :h], in0=mt[:, :h], in1=mt[:, h:w], op=OP.max
                    )
                    w = h
                # m now lives in mt[:, :T]

                # cnt[p, (k, j)] = #t with gathered target row hitting the max
                eq = sp.tile([128, L * T], f32, tag="eq")
                nc.vector.tensor_tensor(
                    out=eq[:, :].rearrange("p (l t) -> p l t", l=L),
                    in0=xga[:, k * L * T:(k + 1) * L * T].rearrange(
                        "p (l t) -> p l t", l=L
                    ),
                    in1=mt[:, :T].unsqueeze(1).to_broadcast([128, L, T]),
                    op=OP.is_equal,
                )
                nc.vector.reduce_sum(
                    out=cnta[:, k * L:(k + 1) * L],
                    in_=eq[:, :].rearrange("p (l t) -> p l t", l=L),
                    axis=AX.X,
                )

            # ---- final epilogue (needs cnta): [128, 64] math ----
            # n_sum[p, k] = sum_j cnt/dup, clamped to >= 1 (cnt==0 there anyway)
            nd = cp.tile([128, NT * L], f32)
            nc.vector.tensor_mul(out=nd[:, :], in0=cnta[:, :], in1=rd[:, :])
            nsum = cp.tile([128, NT], f32)
            nc.vector.reduce_sum(
                out=nsum[:, :],
                in_=nd[:, :].rearrange("p (k j) -> p k j", j=L),
                axis=AX.X,
            )
            nc.vector.tensor_scalar_max(out=nsum[:, :], in0=nsum[:, :], scalar1=1.0)
            inv = cp.tile([128, NT], f32)
            nc.vector.reciprocal(out=inv[:, :], in_=nsum[:, :])
            # n_p = max(cnt * inv, EPS); loss_j = n_p * wgt
            npj = cp.tile([128, NT * L], f32)
            nc.vector.tensor_tensor(
                out=npj[:, :].rearrange("p (k j) -> p k j", j=L),
                in0=cnta[:, :].rearrange("p (k j) -> p k j", j=L),
                in1=inv[:, :].unsqueeze(2).to_broadcast([128, NT, L]),
                op=OP.mult,
            )
            nc.vector.tensor_scalar_max(out=npj[:, :], in0=npj[:, :], scalar1=EPS)
            lj = cp.tile([128, NT * L], f32)
            nc.vector.tensor_mul(out=lj[:, :], in0=npj[:, :], in1=wgt[:, :])
            acc = cp.tile([128, 1], f32)
            nc.vector.reduce_sum(
                out=acc[:, :],
                in_=lj[:, :].rearrange("p (k j) -> p k j", j=L),
                axis=AX.XY,
            )
            nc.sync.dma_start(out=out[:, :], in_=acc[:, :])
    nc.compile()
    return nc


def _shard_inputs(x, y, target_lengths):
    """Numpy-side sharding, target-row pre-gather, and device layouts."""
    import os
    gdt = (np.float32 if "f32tree" in os.environ.get("ACE_VARIANT", "")
           else np.float16)
    x = np.ascontiguousarray(np.asarray(x, dtype=np.float32))
    y = np.asarray(y, dtype=np.int32)
    y2 = y.reshape(B, L)  # target_lengths is L for every sample (spec'd)
    x3 = x.reshape(B, C, T)
    # gathered target rows for all samples, in the compare dtype: [B, L, T]
    xg_all = np.take_along_axis(
        x3, y2[:, :, None].astype(np.int64), axis=1
    ).astype(gdt)

    in_maps = []
    for i in range(N_CORES):
        sl = slice(i * B_SH, (i + 1) * B_SH)
        xs = x[sl].reshape(B_SH, C * T)
        # [p, (tile, slot, t)] and [p, (tile, slot)] layouts
        xgs = np.ascontiguousarray(
            xg_all[sl].reshape(NT, 128, L * T).transpose(1, 0, 2).reshape(128, -1)
        )
        ycs = np.ascontiguousarray(
            y2[sl].reshape(NT, 128, L).transpose(1, 0, 2).reshape(128, -1)
        )
        in_maps.append({"x": xs, "xg": xgs, "yc": ycs})
    return in_maps


def kernel(x, y, target_lengths):
    import sys
    if "/opt/trn_rl_repo" not in sys.path:
        sys.path.insert(0, "/opt/trn_rl_repo")
    from concourse.bass_utils import run_bass_kernel_spmd

    if "nc" not in _CACHE:
        _CACHE["nc"] = _build_nc()
    nc = _CACHE["nc"]

    in_maps = _shard_inputs(x, y, target_lengths)
    res = run_bass_kernel_spmd(nc, in_maps, core_ids=list(range(N_CORES)))
    total = np.float64(0.0)
    for r in res.results:
        total += np.asarray(r["out"], dtype=np.float64).sum()
    return np.float32(total / B)
